# revision 1
# baseline (speedup 1.0000x reference)
"""Bass/Trainium2 kernel for nn_BernoulliMixture.

Reference computation (fp32):
    h = leaky_relu(x @ W_i2h^T + b_i2h)              [4096, 1024]
    z = softmax(h @ W_h2z^T + b_h2z)                 [4096, 32]
    d = sigmoid((h @ W_h2d^T + b_h2d) -> [.., 32, 784])
    out = einsum('tk,tko->to', z, d)                 [4096, 784]

Sharding (8 cores, SPMD): 4 token groups x 2 component groups.
Each core handles 1024 tokens and 16 of the 32 mixture components
(its W_h2d shard is [1024, 16*784]); it computes the full 32-way
softmax locally (w_h2z is tiny and gets its columns permuted per-core
so the core's own 16 components sit in columns 0..15 — the program
stays identical across cores). Each core emits the partial mixture sum
over its components, already scaled by 1/sum(exp(zlog)); the host adds
the two component-group partials per token group.

On-chip layout is token-major [128 tokens (partitions), free]:
  - h^T tiles [128 j, t] feed the PE as the stationary operand,
  - d-logits come out of PSUM as [128 t, r] so the free-dim bias add,
    sigmoid, and the per-partition-scalar K-reduction (fused DVE
    scalar_tensor_tensor: U += E_k * D) need no partition reductions.
Matmuls run as float32r (full PE rate with moving dim >= 256).
"""

import os
from contextlib import ExitStack

import numpy as np

# ---------------------------------------------------------------------------
# problem constants (hardcoded; kernel.py must be self-contained)
B, L, IN, HID, K, O = 4, 1024, 512, 1024, 32, 784
N_CORES = 8
TOK_GROUPS = 8          # token-parallel
COMP_GROUPS = 1         # component-parallel
T = (B * L) // TOK_GROUPS          # 1024 tokens per core
CK = K // COMP_GROUPS              # 16 components per core
R = CK * O                          # 12544 d-columns per core
# d-matmul psum windows: 1024-wide (2 PSUM banks) for the bulk, tapered at
# the end so the PE->DVE pipeline drains with less backlog
WIN_PLAN = [1024] * (R // 1024 - 2)
_rest = R - sum(WIN_PLAN)
while _rest > 512:
    WIN_PLAN.append(512)
    _rest -= 512
while _rest:
    WIN_PLAN.append(256)
    _rest -= 256
assert sum(WIN_PLAN) == R
WIN_OFF = [sum(WIN_PLAN[:i]) for i in range(len(WIN_PLAN))]
N_WIN = len(WIN_PLAN)
TCHUNKS = T // 128                  # 8
JC = HID // 128                     # 8 contraction chunks of h
IC = IN // 128                      # 4 contraction chunks of x

_PROGRAM = None


def _install_drain_patch():
    """This image's walrus accepts at most ONE sync wait on CTRL-class
    instructions (Drain/NoOp). Stock Tile puts one wait per outstanding
    semaphore on the kernel-tail drain; split the extras into a chain of
    single-wait NOPs."""
    import concourse.tile as tile
    import concourse.mybir as mybir

    if getattr(tile.TileContext, "_drain_patch_installed", False):
        return

    def _drain_and_barrier(self, tick_clock, wait_clock):
        nc = self.nc
        drain_inst = nc.sync.drain()
        wait_clock.add_sem_waits(
            drain_inst.ins, tile.ScopedClock({None: tick_clock.global_clock})
        )
        si = drain_inst.ins.sync_info
        waits = list(si.on_wait or []) if si is not None else []
        if len(waits) > 1:
            si.on_wait = waits[:1]
            for w in waits[1:]:
                nop = nc.sync.nop()
                nop.ins.sync_info = mybir.SyncInfo(on_wait=[w], on_update=[])

        nc.all_engine_barrier()
        assert self.sems is not None
        popped = nc._tile_sem_poison_stack.pop()
        assert popped is self._sem_poison
        nc.clear_and_free_semaphores(list(self.sems.allocated().values()))
        nc.all_engine_barrier()

    tile.TileContext._drain_and_barrier = _drain_and_barrier
    tile.TileContext._drain_patch_installed = True


def _legalize_waits(nc):
    """This image's walrus accepts at most ONE sync wait per instruction.
    Hoist extra waits into preceding single-wait NOPs on the same engine
    (engines execute their stream in order, so a prior NOP-wait gates the
    instruction identically)."""
    import concourse.mybir as mybir

    n = 0
    for bass_bb in nc.bb_map.values():
        insts = bass_bb.bb.instructions
        i = 0
        while i < len(insts):
            inst = insts[i]
            si = inst.sync_info
            waits = list(si.on_wait) if si is not None and si.on_wait else []
            if len(waits) > 1:
                for w in waits[:-1]:
                    nop = mybir.InstNoOp(
                        name=f"waitnop_{n}", engine=inst.engine, ins=[], outs=[],
                        sync_info=mybir.SyncInfo(on_wait=[w], on_update=[]),
                    )
                    n += 1
                    insts.insert(i, nop)
                    i += 1
                si.on_wait = waits[-1:]
            i += 1
    return n


def _d_segments(w0, w1):
    """(kk, s0, s1) pieces of dram-column range [w0, w1) split at component
    boundaries (784 columns per component)."""
    segs = []
    for kk in range(w0 // O, (w1 - 1) // O + 1):
        s0, s1 = max(w0, kk * O), min(w1, (kk + 1) * O)
        segs.append((kk, s0, s1))
    return segs


def _build_program():
    import concourse.bass as bass
    import concourse.mybir as mybir
    import concourse.tile as tile

    _install_drain_patch()
    f32 = mybir.dt.float32
    f32r = mybir.dt.float32r
    bf16 = mybir.dt.bfloat16
    AF = mybir.ActivationFunctionType
    ALU = mybir.AluOpType

    nc = bass.Bass("TRN2", target_bir_lowering=False, debug=False,
                   num_devices=N_CORES)

    d_xT = nc.dram_tensor("xT", [IC, 128, T], bf16, kind="ExternalInput").ap()
    d_wi2hT = nc.dram_tensor("wi2hT", [IC, 128, HID], bf16, kind="ExternalInput").ap()
    d_bi2h = nc.dram_tensor("bi2h", [128, JC], f32, kind="ExternalInput").ap()
    d_bneg = nc.dram_tensor("bneg", [128, JC], f32, kind="ExternalInput").ap()
    d_wzT = nc.dram_tensor("wzT", [128, JC, K], bf16, kind="ExternalInput").ap()
    d_bz = nc.dram_tensor("bz", [1, K], f32, kind="ExternalInput").ap()
    d_wdT = nc.dram_tensor("wdT", [128, JC, R], bf16, kind="ExternalInput").ap()
    d_bd = nc.dram_tensor("bd", [128, R], f32, kind="ExternalInput").ap()
    d_out = nc.dram_tensor("out", [T, O], f32, kind="ExternalOutput").ap()

    with tile.TileContext(nc) as tc:
        with (
            tc.tile_pool(name="consts", bufs=1) as consts,
            tc.tile_pool(name="hpool", bufs=1) as hpool,
            tc.tile_pool(name="upool", bufs=1) as upool,
            tc.tile_pool(name="epool", bufs=1) as epool,
            tc.tile_pool(name="tmp", bufs=2) as tmp,
        ):

            # ---- phase H: h^T[j, t] = leaky_relu(x W^T + b) ----------------
            h_sb = [hpool.tile([128, T], bf16, tag=f"h{j}", name=f"h{j}")
                    for j in range(JC)]
            hzctx = ExitStack()
            hz_psum = hzctx.enter_context(
                tc.tile_pool(name="hz_psum", bufs=4, space="PSUM"))
            esc_sb = [None] * TCHUNKS
            dctx = ExitStack()
            wslab_pool = dctx.enter_context(tc.tile_pool(name="wslab", bufs=6))
            bslab_pool = dctx.enter_context(tc.tile_pool(name="bslab", bufs=2))
            dtmp = dctx.enter_context(tc.tile_pool(name="dtmp", bufs=3))

            def load_slabs(w):
                w0 = WIN_OFF[w]
                win = WIN_PLAN[w]
                w1 = w0 + win
                wsls = []
                for sub in range(0, win, 512):
                    sw = min(512, win - sub)
                    wsl = wslab_pool.tile([128, JC, sw], bf16, tag="w",
                                          name=f"wsl{w}_{sub}")
                    for ja in range(0, JC, 2):
                        nc.sync.dma_start(
                            wsl[:, ja:ja + 2, :],
                            d_wdT[:, ja:ja + 2, w0 + sub:w0 + sub + sw])
                    wsls.append((sub, sw, wsl))
                bsl = bslab_pool.tile([128, win], f32, tag="b", name=f"bsl{w}")
                half = win // 2
                nc.scalar.dma_start(bsl[:, 0:half], d_bd[:, w0:w0 + half])
                nc.scalar.dma_start(bsl[:, half:win], d_bd[:, w0 + half:w1])
                return wsls, bsl

            with (
                tc.tile_pool(name="xw", bufs=1) as xw,
            ):
                x_sb, wi_sb = [], []
                for i in range(IC):
                    xt = xw.tile([128, T], bf16, tag=f"x{i}", name=f"x{i}")
                    x_sb.append(xt)
                    wt = xw.tile([128, HID], bf16, tag=f"wi{i}", name=f"wi{i}")
                    wi_sb.append(wt)
                # split the loads so the first matmul's operands land first
                # (one dma_start = one HW queue; fine pieces spread queues)
                for i in range(IC):
                    nc.sync.dma_start(wi_sb[i][:, 0:128], d_wi2hT[i][:, 0:128])
                    nc.scalar.dma_start(x_sb[i][:, 0:256], d_xT[i][:, 0:256])
                    nc.sync.dma_start(x_sb[i][:, 256:512], d_xT[i][:, 256:512])
                # constants ride the scalar-engine DMA queues, off the
                # critical SP dispatch path
                bi2h_sb = consts.tile([128, JC], f32)
                nc.scalar.dma_start(bi2h_sb[:], d_bi2h[:])
                bneg_sb = consts.tile([128, JC], f32)
                nc.scalar.dma_start(bneg_sb[:], d_bneg[:])
                wz_sb = consts.tile([128, JC, K], bf16)
                nc.scalar.dma_start(wz_sb[:], d_wzT[:])
                bz_sb = consts.tile([1, K], f32)
                nc.scalar.dma_start(bz_sb[:], d_bz[:])
                ones_sb = consts.tile([1, 128], f32)
                nc.vector.memset(ones_sb[:], 1.0)
                u_sb = []
                for t in range(TCHUNKS):
                    u = upool.tile([128, O], f32, tag=f"u{t}", name=f"u{t}")
                    nc.vector.memset(u[:], 0.0)
                    u_sb.append(u)
                for i in range(IC):
                    for n4, (c0, c1) in enumerate(((128, 512), (512, 768),
                                                   (768, HID))):
                        eng = nc.scalar if n4 % 2 else nc.sync
                        eng.dma_start(wi_sb[i][:, c0:c1], d_wi2hT[i][:, c0:c1])
                    if T > 512:
                        nc.scalar.dma_start(x_sb[i][:, 512:T],
                                            d_xT[i][:, 512:T])
                preloaded = {w: load_slabs(w) for w in range(2)}

                # H and Z interleaved: after each 512-token half of h is
                # done, immediately compute that half's softmax numerators
                for tw in range(T // 512):
                    for j in range(JC):
                        ph = hz_psum.tile([128, 512], f32, tag="ph")
                        for i in range(IC):
                            nc.tensor.matmul(
                                ph[:],
                                lhsT=wi_sb[i][:, j * 128:(j + 1) * 128],
                                rhs=x_sb[i][:, tw * 512:(tw + 1) * 512],
                                start=(i == 0),
                                stop=(i == IC - 1),
                            )
                        r1 = xw.tile([128, 512], f32, tag="r1", bufs=2,
                                     name=f"r1_{tw}_{j}")
                        nc.scalar.activation(r1[:], ph[:], AF.Relu,
                                             bias=bi2h_sb[:, j:j + 1], scale=1.0)
                        r2 = xw.tile([128, 512], f32, tag="r2", bufs=2,
                                     name=f"r2_{tw}_{j}")
                        nc.scalar.activation(r2[:], ph[:], AF.Relu,
                                             bias=bneg_sb[:, j:j + 1], scale=-1.0)
                        # h = r1 - 0.01*r2  (leaky relu)
                        nc.vector.scalar_tensor_tensor(
                            out=h_sb[j][:, tw * 512:(tw + 1) * 512],
                            in0=r2[:], scalar=-0.01, in1=r1[:],
                            op0=ALU.mult, op1=ALU.add,
                        )
                    for t in range(tw * 4, tw * 4 + 4):
                        pz = hz_psum.tile([128, K], f32, tag="pz",
                                          name=f"pz{t}")
                        for j in range(JC):
                            nc.tensor.matmul(
                                pz[:],
                                lhsT=h_sb[j][:, t * 128:(t + 1) * 128],
                                rhs=wz_sb[:, j, :],
                                start=(j == 0),
                                stop=False,
                            )
                        # + b_h2z via rank-1 update: ones[t] x bz
                        nc.tensor.matmul(
                            pz[:],
                            lhsT=ones_sb[:],
                            rhs=bz_sb[:],
                            start=False,
                            stop=True,
                        )
                        e_t = epool.tile([128, K], f32, tag=f"e{t}",
                                         name=f"e{t}")
                        s_t = tmp.tile([128, 1], f32, tag="s", name=f"s{t}")
                        nc.scalar.activation(e_t[:], pz[:], AF.Exp,
                                             accum_out=s_t[:])
                        sinv = tmp.tile([128, 1], f32, tag="sinv",
                                        name=f"sinv{t}")
                        nc.vector.reciprocal(sinv[:], s_t[:])
                        esc = epool.tile([128, K], f32, tag=f"esc{t}",
                                         name=f"esc{t}")
                        nc.vector.tensor_scalar(esc[:], e_t[:], sinv[:], None,
                                                ALU.mult)
                        esc_sb[t] = esc

            # ---- phase D: stream W shard, accumulate U ---------------------
            hzctx.close()
            pctx = ExitStack()
            d_psum = pctx.enter_context(
                tc.tile_pool(name="d_psum", bufs=4, space="PSUM"))
            for w in range(N_WIN):
                w0 = WIN_OFF[w]
                win = WIN_PLAN[w]
                w1 = w0 + win
                wsls, bsl = preloaded.pop(w) if w in preloaded else load_slabs(w)
                segs = _d_segments(w0, w1)
                t_order = range(TCHUNKS)
                if w == N_WIN - 1:
                    t_order = reversed(range(TCHUNKS))
                for t in t_order:
                    pd = d_psum.tile([128, win], f32, tag="pd", name=f"pd{w}_{t}")
                    # j outer / sub inner: both 512-subs reuse the same
                    # stationary h tile, halving weight loads
                    for j in range(JC):
                        for sub, sw, wsl in wsls:
                            nc.tensor.matmul(
                                pd[:, sub:sub + sw],
                                lhsT=h_sb[j][:, t * 128:(t + 1) * 128],
                                rhs=wsl[:, j, :],
                                start=(j == 0),
                                stop=(j == JC - 1),
                            )
                    db = dtmp.tile([128, win], f32, tag="db")
                    nc.vector.tensor_tensor(db[:], pd[:], bsl[:], ALU.add)
                    ds = dtmp.tile([128, win], f32, tag="ds")
                    nc.scalar.activation(ds[:], db[:], AF.Sigmoid)
                    for kk, s0, s1 in segs:
                        nc.vector.scalar_tensor_tensor(
                            out=u_sb[t][:, s0 - kk * O:s1 - kk * O],
                            in0=ds[:, s0 - w0:s1 - w0],
                            scalar=esc_sb[t][:, kk:kk + 1],
                            in1=u_sb[t][:, s0 - kk * O:s1 - kk * O],
                            op0=ALU.mult, op1=ALU.add,
                        )

            for t in reversed(range(TCHUNKS)):
                nc.scalar.dma_start(d_out[t * 128:(t + 1) * 128, 0:392],
                                  u_sb[t][:, 0:392])
                nc.scalar.dma_start(d_out[t * 128:(t + 1) * 128, 392:O],
                                  u_sb[t][:, 392:O])
            pctx.close()
            dctx.close()

    _legalize_waits(nc)
    return nc


def _get_program():
    global _PROGRAM
    if _PROGRAM is None:
        _PROGRAM = _build_program()
    return _PROGRAM


def _prep_inputs(input, w_i2h, b_i2h, w_h2z, b_h2z, w_h2d, b_h2d):
    """Build the 8 per-core in_maps (host-side transposes/shards)."""
    import ml_dtypes
    x_flat = np.ascontiguousarray(input.reshape(B * L, IN).astype(np.float32))
    wi2hT = np.ascontiguousarray(
        w_i2h.astype(np.float32).T.reshape(IC, 128, HID)
    ).astype(ml_dtypes.bfloat16)
    bi = np.ascontiguousarray(b_i2h.astype(np.float32).reshape(JC, 128).T)
    bn = np.ascontiguousarray(-bi)

    wzT_full = w_h2z.astype(np.float32).T          # [HID, K]
    bz_full = b_h2z.astype(np.float32)
    wdT_full = w_h2d.astype(np.float32).T          # [HID, K*O]
    bd_full = b_h2d.astype(np.float32)

    per_cg = {}
    for cg in range(COMP_GROUPS):
        # permute z columns: own components first
        own = list(range(cg * CK, (cg + 1) * CK))
        rest = [k for k in range(K) if k not in own]
        perm = own + rest
        import ml_dtypes
        wz = np.ascontiguousarray(
            wzT_full[:, perm].reshape(JC, 128, K).transpose(1, 0, 2)
        ).astype(ml_dtypes.bfloat16)
        bz = np.ascontiguousarray(bz_full[perm].reshape(1, K))
        wd = np.ascontiguousarray(
            wdT_full[:, cg * R:(cg + 1) * R]
            .reshape(JC, 128, R).transpose(1, 0, 2)
        ).astype(ml_dtypes.bfloat16)
        bd = np.ascontiguousarray(
            np.broadcast_to(bd_full[cg * R:(cg + 1) * R], (128, R)))
        per_cg[cg] = (wz, bz, wd, bd)

    in_maps = []
    for core in range(N_CORES):
        tg, cg = core // COMP_GROUPS, core % COMP_GROUPS
        xT = np.ascontiguousarray(
            x_flat[tg * T:(tg + 1) * T, :].T.reshape(IC, 128, T)
        ).astype(ml_dtypes.bfloat16)
        wz, bz, wd, bd = per_cg[cg]
        in_maps.append({
            "xT": xT, "wi2hT": wi2hT, "bi2h": bi, "bneg": bn,
            "wzT": wz, "bz": bz, "wdT": wd, "bd": bd,
        })
    return in_maps


LAST_RESULT = None


def kernel(**inputs):
    from concourse.bass_utils import run_bass_kernel_spmd

    global LAST_RESULT
    nc = _get_program()
    in_maps = _prep_inputs(**inputs)
    trace = bool(os.environ.get("BASS_KERNEL_TRACE"))
    if trace:
        try:
            _install_profile_shim()
        except Exception as e:  # degrade to untraced run
            print(f"profile shim unavailable ({e}); running untraced")
            trace = False
    res = run_bass_kernel_spmd(nc, in_maps, list(range(N_CORES)), trace=trace)
    LAST_RESULT = res

    out = np.empty((B * L, O), dtype=np.float32)
    for tg in range(TOK_GROUPS):
        acc = res.results[tg * COMP_GROUPS + 0]["out"].astype(np.float32)
        for cg in range(1, COMP_GROUPS):
            acc = acc + res.results[tg * COMP_GROUPS + cg]["out"]
        out[tg * T:(tg + 1) * T] = acc
    return out.reshape(B, L, O)


def _install_profile_shim():
    """Register the NTFF profile hook concourse expects under axon (the
    image's antenv lacks axon_hooks) and stub the artifact upload."""
    import sys
    import types

    if "antenv.axon_hooks" not in sys.modules:
        from trn_agent_boot.trn_boot import _ntff_profile_via_ctypes

        hook = _ntff_profile_via_ctypes("/opt/axon/libaxon_pjrt.so")
        m = types.ModuleType("antenv.axon_hooks")
        m.get_axon_ntff_profile_hook = lambda: hook
        m.set_axon_ntff_profile_hook = lambda h: None
        sys.modules["antenv.axon_hooks"] = m

    import concourse.bass_utils as bu

    bu.upload_artifacts = lambda tmpdir: f"local://{tmpdir}"



# revision 13
# speedup vs baseline: 1.1307x; 1.1307x over previous
"""Bass/Trainium2 kernel for nn_BernoulliMixture.

Reference computation (fp32):
    h = leaky_relu(x @ W_i2h^T + b_i2h)              [4096, 1024]
    z = softmax(h @ W_h2z^T + b_h2z)                 [4096, 32]
    d = sigmoid((h @ W_h2d^T + b_h2d) -> [.., 32, 784])
    out = einsum('tk,tko->to', z, d)                 [4096, 784]

Sharding (8 cores, SPMD): data-parallel over tokens. Each core handles
512 tokens and the full 32-component mixture.

The d-matmul (98% of all PE work) runs in fp8 e4m3 with
perf_mode=DoubleRow: both operands are [128, 2, N] APs pairing two
128-row contraction chunks per instruction, so the PE streams 2
elements/cycle — half the bf16 streaming cycles. Scales (h x4, w x32)
are folded into the host-prepped bias (x128) and the sigmoid
activation's scale (1/128), so no extra rescale pass is needed.
Numerics: max rel err ~1.5e-2 vs the 2e-2 gate (validated in numpy).

On-chip layout is token-major [128 tokens (partitions), free]:
  - h^T fp8 pair tiles [128 j, 2, t] are the stationary operand,
  - d-logits land in PSUM as [128 t, win]; bias-add (Vector
    tensor_tensor vs a bf16 bias slab), sigmoid (Scalar, scale=1/128),
    and the per-partition-scalar K-reduction (scalar_tensor_tensor:
    U += E_k * D) need no partition reductions.
  - ds and U are fp16 so the K-reduction STT runs in the DVE's 2x
    16-bit mode; for a subset of windows (PEB_WINS) the bias rides a
    rank-1 fp8 matmul into PSUM instead of a Vector tensor_tensor,
    balancing Vector against the fp8 PE time. U converts to fp32 on
    the host.
"""

import os
from contextlib import ExitStack

import numpy as np

# ---------------------------------------------------------------------------
# problem constants (hardcoded; kernel.py must be self-contained)
B, L, IN, HID, K, O = 4, 1024, 512, 1024, 32, 784
N_CORES = 8
TOK_GROUPS = 8          # token-parallel
T = (B * L) // TOK_GROUPS          # 512 tokens per core
R = K * O                           # 25088 d-columns per core
TCHUNKS = T // 128                  # 4
JC = HID // 128                     # 8 contraction chunks of h
JP = JC // 2                        # 4 DoubleRow chunk pairs
IC = IN // 128                      # 4 contraction chunks of x

SH = 4.0                            # fp8 scale on h
SWD = 32.0                          # fp8 scale on w_h2d
INV_S = 1.0 / (SH * SWD)            # folded into the sigmoid activation

# d-matmul psum windows: 2048-wide (4 PSUM banks, 2 in flight) for the
# bulk, with a 512 taper so the PE->DVE pipeline drains with less backlog
WIN_PLAN = [2048] * (R // 2048) + [R - 2048 * (R // 2048)]
assert sum(WIN_PLAN) == R and WIN_PLAN[-1] == 512
WIN_OFF = [sum(WIN_PLAN[:i]) for i in range(len(WIN_PLAN))]
N_WIN = len(WIN_PLAN)

SB = 1024.0                         # fp8 scale on b_h2d (PE-bias windows)
# windows whose bias rides a rank-1 fp8 matmul into PSUM (start=True on a
# (1/16*ones, b8) DoubleRow pair) instead of a Vector tensor_tensor --
# shifts bias-add work from the DVE to the PE to balance the two engines
PEB_WINS = frozenset((2, 5, 8, 11))

_PROGRAM = None


def _install_drain_patch():
    """This image's walrus accepts at most ONE sync wait on CTRL-class
    instructions (Drain/NoOp). Stock Tile puts one wait per outstanding
    semaphore on the kernel-tail drain; split the extras into a chain of
    single-wait NOPs."""
    import concourse.tile as tile
    import concourse.mybir as mybir

    if getattr(tile.TileContext, "_drain_patch_installed", False):
        return

    def _drain_and_barrier(self, tick_clock, wait_clock):
        nc = self.nc
        drain_inst = nc.sync.drain()
        wait_clock.add_sem_waits(
            drain_inst.ins, tile.ScopedClock({None: tick_clock.global_clock})
        )
        si = drain_inst.ins.sync_info
        waits = list(si.on_wait or []) if si is not None else []
        if len(waits) > 1:
            si.on_wait = waits[:1]
            for w in waits[1:]:
                nop = nc.sync.nop()
                nop.ins.sync_info = mybir.SyncInfo(on_wait=[w], on_update=[])

        nc.all_engine_barrier()
        assert self.sems is not None
        popped = nc._tile_sem_poison_stack.pop()
        assert popped is self._sem_poison
        nc.clear_and_free_semaphores(list(self.sems.allocated().values()))
        nc.all_engine_barrier()

    tile.TileContext._drain_and_barrier = _drain_and_barrier
    tile.TileContext._drain_patch_installed = True


def _legalize_waits(nc):
    """This image's walrus accepts at most ONE sync wait per instruction.
    Hoist extra waits into preceding single-wait NOPs on the same engine
    (engines execute their stream in order, so a prior NOP-wait gates the
    instruction identically)."""
    import concourse.mybir as mybir

    n = 0
    for bass_bb in nc.bb_map.values():
        insts = bass_bb.bb.instructions
        i = 0
        while i < len(insts):
            inst = insts[i]
            si = inst.sync_info
            waits = list(si.on_wait) if si is not None and si.on_wait else []
            if len(waits) > 1:
                for w in waits[:-1]:
                    nop = mybir.InstNoOp(
                        name=f"waitnop_{n}", engine=inst.engine, ins=[], outs=[],
                        sync_info=mybir.SyncInfo(on_wait=[w], on_update=[]),
                    )
                    n += 1
                    insts.insert(i, nop)
                    i += 1
                si.on_wait = waits[-1:]
            i += 1
    return n


def _d_segments(w0, w1):
    """(kk, s0, s1) pieces of dram-column range [w0, w1) split at component
    boundaries (784 columns per component)."""
    segs = []
    for kk in range(w0 // O, (w1 - 1) // O + 1):
        s0, s1 = max(w0, kk * O), min(w1, (kk + 1) * O)
        segs.append((kk, s0, s1))
    return segs


def _build_program():
    import concourse.bass as bass
    import concourse.mybir as mybir
    import concourse.tile as tile

    _install_drain_patch()
    f32 = mybir.dt.float32
    f16 = mybir.dt.float16
    bf16 = mybir.dt.bfloat16
    f8 = mybir.dt.float8e4
    AF = mybir.ActivationFunctionType
    ALU = mybir.AluOpType
    DR = mybir.MatmulPerfMode.DoubleRow

    nc = bass.Bass("TRN2", target_bir_lowering=False, debug=False,
                   num_devices=N_CORES)

    d_xT = nc.dram_tensor("xT", [IC, 128, T], bf16, kind="ExternalInput").ap()
    d_wi2hT = nc.dram_tensor("wi2hT", [IC, 128, HID], bf16, kind="ExternalInput").ap()
    d_bi2h = nc.dram_tensor("bi2h", [128, JC], f32, kind="ExternalInput").ap()
    d_bneg = nc.dram_tensor("bneg", [128, JC], f32, kind="ExternalInput").ap()
    d_wzT = nc.dram_tensor("wzT", [128, JC, K], bf16, kind="ExternalInput").ap()
    d_bz = nc.dram_tensor("bz", [1, K], f32, kind="ExternalInput").ap()
    d_wdT = nc.dram_tensor("wdT", [128, JP, 2, R], f8, kind="ExternalInput").ap()
    d_bd = nc.dram_tensor("bd", [128, R], bf16, kind="ExternalInput").ap()
    d_b8 = nc.dram_tensor("b8", [1, 2, R], f8, kind="ExternalInput").ap()
    d_ones8 = nc.dram_tensor("ones8", [1, 2, 128], f8, kind="ExternalInput").ap()
    d_out = nc.dram_tensor("out", [T, O], f16, kind="ExternalOutput").ap()

    with tile.TileContext(nc) as tc:
        with (
            tc.tile_pool(name="consts", bufs=1) as consts,
            tc.tile_pool(name="hpool", bufs=1) as hpool,
            tc.tile_pool(name="upool", bufs=1) as upool,
            tc.tile_pool(name="epool", bufs=1) as epool,
            tc.tile_pool(name="tmp", bufs=2) as tmp,
        ):

            # ---- phase H: h^T[j, t] = leaky_relu(x W^T + b) ----------------
            h_sb = [hpool.tile([128, T], bf16, tag=f"h{j}", name=f"h{j}")
                    for j in range(JC)]
            h8 = [hpool.tile([128, 2, T], f8, tag=f"h8_{jp}", name=f"h8_{jp}")
                  for jp in range(JP)]
            hzctx = ExitStack()
            hz_psum = hzctx.enter_context(
                tc.tile_pool(name="hz_psum", bufs=4, space="PSUM"))
            esc_sb = [None] * TCHUNKS
            dctx = ExitStack()
            wslab_pool = dctx.enter_context(tc.tile_pool(name="wslab", bufs=3))
            bslab_pool = dctx.enter_context(tc.tile_pool(name="bslab", bufs=2))
            dtmp = dctx.enter_context(tc.tile_pool(name="dtmp", bufs=3))

            def load_slabs(w):
                w0 = WIN_OFF[w]
                win = WIN_PLAN[w]
                w1 = w0 + win
                wsl = wslab_pool.tile([128, JP, 2, win], f8, tag="w",
                                      name=f"wsl{w}")
                for jp in range(JP):
                    nc.sync.dma_start(wsl[:, jp, :, :],
                                      d_wdT[:, jp, :, w0:w1])
                if w in PEB_WINS:
                    bsl = bslab_pool.tile([1, 2, win], f8, tag="b8",
                                          name=f"b8sl{w}")
                    nc.sync.dma_start(bsl[:], d_b8[:, :, w0:w1])
                else:
                    bsl = bslab_pool.tile([128, win], bf16, tag="b",
                                          name=f"bsl{w}")
                    half = win // 2
                    nc.sync.dma_start(bsl[:, 0:half], d_bd[:, w0:w0 + half])
                    nc.sync.dma_start(bsl[:, half:win], d_bd[:, w0 + half:w1])
                return wsl, bsl

            with (
                tc.tile_pool(name="xw", bufs=1) as xw,
            ):
                x_sb, wi_sb = [], []
                for i in range(IC):
                    xt = xw.tile([128, T], bf16, tag=f"x{i}", name=f"x{i}")
                    x_sb.append(xt)
                    wt = xw.tile([128, HID], bf16, tag=f"wi{i}", name=f"wi{i}")
                    wi_sb.append(wt)
                # split the loads so the first matmul's operands land first
                # (one dma_start = one HW queue; fine pieces spread queues)
                for i in range(IC):
                    nc.sync.dma_start(wi_sb[i][:, 0:128], d_wi2hT[i][:, 0:128])
                    nc.scalar.dma_start(x_sb[i][:, 0:256], d_xT[i][:, 0:256])
                    nc.sync.dma_start(x_sb[i][:, 256:512], d_xT[i][:, 256:512])
                # constants ride the scalar-engine DMA queues, off the
                # critical SP dispatch path
                bi2h_sb = consts.tile([128, JC], f32)
                nc.scalar.dma_start(bi2h_sb[:], d_bi2h[:])
                bneg_sb = consts.tile([128, JC], f32)
                nc.scalar.dma_start(bneg_sb[:], d_bneg[:])
                wz_sb = consts.tile([128, JC, K], bf16)
                nc.scalar.dma_start(wz_sb[:], d_wzT[:])
                bz_sb = consts.tile([1, K], f32)
                nc.scalar.dma_start(bz_sb[:], d_bz[:])
                ones_sb = consts.tile([1, 128], f32)
                nc.vector.memset(ones_sb[:], 1.0)
                ones8_sb = consts.tile([1, 2, 128], f8)
                nc.scalar.dma_start(ones8_sb[:], d_ones8[:])
                u_sb = []
                for t in range(TCHUNKS):
                    u = upool.tile([128, O], f16, tag=f"u{t}", name=f"u{t}")
                    nc.vector.memset(u[:], 0.0)
                    u_sb.append(u)
                for i in range(IC):
                    for n4, (c0, c1) in enumerate(((128, 512), (512, 768),
                                                   (768, HID))):
                        eng = nc.scalar if n4 % 2 else nc.sync
                        eng.dma_start(wi_sb[i][:, c0:c1], d_wi2hT[i][:, c0:c1])
                    if T > 512:
                        nc.scalar.dma_start(x_sb[i][:, 512:T],
                                            d_xT[i][:, 512:T])
                preloaded = {w: load_slabs(w) for w in range(2)}

                # H and Z interleaved: after each 512-token half of h is
                # done, immediately compute that half's softmax numerators
                for tw in range(T // 512):
                    for j in range(JC):
                        ph = hz_psum.tile([128, 512], f32, tag="ph")
                        for i in range(IC):
                            nc.tensor.matmul(
                                ph[:],
                                lhsT=wi_sb[i][:, j * 128:(j + 1) * 128],
                                rhs=x_sb[i][:, tw * 512:(tw + 1) * 512],
                                start=(i == 0),
                                stop=(i == IC - 1),
                            )
                        r1 = xw.tile([128, 512], f32, tag="r1", bufs=2,
                                     name=f"r1_{tw}_{j}")
                        nc.scalar.activation(r1[:], ph[:], AF.Relu,
                                             bias=bi2h_sb[:, j:j + 1], scale=1.0)
                        r2 = xw.tile([128, 512], f32, tag="r2", bufs=2,
                                     name=f"r2_{tw}_{j}")
                        nc.scalar.activation(r2[:], ph[:], AF.Relu,
                                             bias=bneg_sb[:, j:j + 1], scale=-1.0)
                        # h = r1 - 0.01*r2  (leaky relu)
                        nc.vector.scalar_tensor_tensor(
                            out=h_sb[j][:, tw * 512:(tw + 1) * 512],
                            in0=r2[:], scalar=-0.01, in1=r1[:],
                            op0=ALU.mult, op1=ALU.add,
                        )
                        # fp8 copy (scaled by SH) in DoubleRow pair layout
                        nc.scalar.activation(
                            h8[j // 2][:, j % 2, tw * 512:(tw + 1) * 512],
                            h_sb[j][:, tw * 512:(tw + 1) * 512],
                            AF.Copy, scale=SH)
                    for t in range(tw * 4, tw * 4 + 4):
                        pz = hz_psum.tile([128, K], f32, tag="pz",
                                          name=f"pz{t}")
                        for j in range(JC):
                            nc.tensor.matmul(
                                pz[:],
                                lhsT=h_sb[j][:, t * 128:(t + 1) * 128],
                                rhs=wz_sb[:, j, :],
                                start=(j == 0),
                                stop=False,
                            )
                        # + b_h2z via rank-1 update: ones[t] x bz
                        nc.tensor.matmul(
                            pz[:],
                            lhsT=ones_sb[:],
                            rhs=bz_sb[:],
                            start=False,
                            stop=True,
                        )
                        e_t = epool.tile([128, K], f32, tag=f"e{t}",
                                         name=f"e{t}")
                        s_t = tmp.tile([128, 1], f32, tag="s", name=f"s{t}")
                        nc.scalar.activation(e_t[:], pz[:], AF.Exp,
                                             accum_out=s_t[:])
                        sinv = tmp.tile([128, 1], f32, tag="sinv",
                                        name=f"sinv{t}")
                        nc.vector.reciprocal(sinv[:], s_t[:])
                        esc = epool.tile([128, K], f16, tag=f"esc{t}",
                                         name=f"esc{t}")
                        nc.vector.tensor_scalar(esc[:], e_t[:], sinv[:], None,
                                                ALU.mult)
                        esc_sb[t] = esc

            # ---- phase D: stream W shard (fp8 DoubleRow), accumulate U -----
            hzctx.close()
            pctx = ExitStack()
            d_psum = pctx.enter_context(
                tc.tile_pool(name="d_psum", bufs=2, space="PSUM"))
            for w in range(N_WIN):
                w0 = WIN_OFF[w]
                win = WIN_PLAN[w]
                w1 = w0 + win
                wsl, bsl = preloaded.pop(w) if w in preloaded else load_slabs(w)
                segs = _d_segments(w0, w1)
                nsub = win // 512
                t_order = range(TCHUNKS)
                if w == N_WIN - 1:
                    t_order = reversed(range(TCHUNKS))
                peb = w in PEB_WINS
                for t in t_order:
                    pd = d_psum.tile([128, win], f32, tag="pd", name=f"pd{w}_{t}")
                    # jp outer / sub inner: all subs reuse the same
                    # stationary h pair, amortizing weight loads
                    if peb:
                        for sub in range(nsub):
                            nc.tensor.matmul(
                                pd[:, sub * 512:(sub + 1) * 512],
                                lhsT=ones8_sb[:],
                                rhs=bsl[:, :, sub * 512:(sub + 1) * 512],
                                start=True,
                                stop=False,
                                perf_mode=DR,
                            )
                    for jp in range(JP):
                        for sub in range(nsub):
                            nc.tensor.matmul(
                                pd[:, sub * 512:(sub + 1) * 512],
                                lhsT=h8[jp][:, :, t * 128:(t + 1) * 128],
                                rhs=wsl[:, jp, :, sub * 512:(sub + 1) * 512],
                                start=(jp == 0 and not peb),
                                stop=(jp == JP - 1),
                                perf_mode=DR,
                            )
                    if peb:
                        sig_in = pd
                    else:
                        sig_in = dtmp.tile([128, win], f32, tag="db")
                        nc.vector.tensor_tensor(sig_in[:], pd[:], bsl[:],
                                                ALU.add)
                    ds = dtmp.tile([128, win], f16, tag="ds")
                    nc.scalar.activation(ds[:], sig_in[:], AF.Sigmoid,
                                         scale=INV_S)
                    for kk, s0, s1 in segs:
                        nc.vector.scalar_tensor_tensor(
                            out=u_sb[t][:, s0 - kk * O:s1 - kk * O],
                            in0=ds[:, s0 - w0:s1 - w0],
                            scalar=esc_sb[t][:, kk:kk + 1],
                            in1=u_sb[t][:, s0 - kk * O:s1 - kk * O],
                            op0=ALU.mult, op1=ALU.add,
                        )

            for t in reversed(range(TCHUNKS)):
                nc.scalar.dma_start(d_out[t * 128:(t + 1) * 128, 0:392],
                                  u_sb[t][:, 0:392])
                nc.scalar.dma_start(d_out[t * 128:(t + 1) * 128, 392:O],
                                  u_sb[t][:, 392:O])
            pctx.close()
            dctx.close()

    _legalize_waits(nc)
    return nc


def _get_program():
    global _PROGRAM
    if _PROGRAM is None:
        _PROGRAM = _build_program()
    return _PROGRAM


def _prep_inputs(input, w_i2h, b_i2h, w_h2z, b_h2z, w_h2d, b_h2d):
    """Build the 8 per-core in_maps (host-side transposes/shards)."""
    import ml_dtypes
    f8 = ml_dtypes.float8_e4m3
    x_flat = np.ascontiguousarray(input.reshape(B * L, IN).astype(np.float32))
    wi2hT = np.ascontiguousarray(
        w_i2h.astype(np.float32).T.reshape(IC, 128, HID)
    ).astype(ml_dtypes.bfloat16)
    bi = np.ascontiguousarray(b_i2h.astype(np.float32).reshape(JC, 128).T)
    bn = np.ascontiguousarray(-bi)

    wz = np.ascontiguousarray(
        w_h2z.astype(np.float32).T.reshape(JC, 128, K).transpose(1, 0, 2)
    ).astype(ml_dtypes.bfloat16)
    bz = np.ascontiguousarray(b_h2z.astype(np.float32).reshape(1, K))

    # w_h2d^T in DoubleRow pair layout [128, JP, 2, R], fp8 e4m3, scaled
    wdT = w_h2d.astype(np.float32).T * np.float32(SWD)     # [HID, R]
    np.clip(wdT, -240.0, 240.0, out=wdT)
    wd = np.ascontiguousarray(
        wdT.reshape(JP, 2, 128, R).transpose(2, 0, 1, 3)
    ).astype(f8)
    bd = np.ascontiguousarray(np.broadcast_to(
        (b_h2d.astype(np.float32) * np.float32(SH * SWD)), (128, R)
    )).astype(ml_dtypes.bfloat16)
    # PE-bias path: psum += 2 * (1/16) * (b*1024) = 128 * b
    b8 = np.ascontiguousarray(np.broadcast_to(
        (b_h2d.astype(np.float32) * np.float32(SB)), (1, 2, R)
    )).astype(f8)
    ones8 = np.full((1, 2, 128), SH * SWD / (2.0 * SB), dtype=f8)

    in_maps = []
    for core in range(N_CORES):
        tg = core
        xT = np.ascontiguousarray(
            x_flat[tg * T:(tg + 1) * T, :].T.reshape(IC, 128, T)
        ).astype(ml_dtypes.bfloat16)
        in_maps.append({
            "xT": xT, "wi2hT": wi2hT, "bi2h": bi, "bneg": bn,
            "wzT": wz, "bz": bz, "wdT": wd, "bd": bd,
            "b8": b8, "ones8": ones8,
        })
    return in_maps


LAST_RESULT = None


def kernel(**inputs):
    from concourse.bass_utils import run_bass_kernel_spmd

    global LAST_RESULT
    nc = _get_program()
    in_maps = _prep_inputs(**inputs)
    trace = bool(os.environ.get("BASS_KERNEL_TRACE"))
    if trace:
        try:
            _install_profile_shim()
        except Exception as e:  # degrade to untraced run
            print(f"profile shim unavailable ({e}); running untraced")
            trace = False
    res = run_bass_kernel_spmd(nc, in_maps, list(range(N_CORES)), trace=trace)
    LAST_RESULT = res

    out = np.empty((B * L, O), dtype=np.float32)
    for tg in range(TOK_GROUPS):
        out[tg * T:(tg + 1) * T] = np.asarray(
            res.results[tg]["out"]).astype(np.float32)
    return out.reshape(B, L, O)


def _install_profile_shim():
    """Register the NTFF profile hook concourse expects under axon (the
    image's antenv lacks axon_hooks) and stub the artifact upload."""
    import sys
    import types

    if "antenv.axon_hooks" not in sys.modules:
        from trn_agent_boot.trn_boot import _ntff_profile_via_ctypes

        hook = _ntff_profile_via_ctypes("/opt/axon/libaxon_pjrt.so")
        m = types.ModuleType("antenv.axon_hooks")
        m.get_axon_ntff_profile_hook = lambda: hook
        m.set_axon_ntff_profile_hook = lambda h: None
        sys.modules["antenv.axon_hooks"] = m

    import concourse.bass_utils as bu

    bu.upload_artifacts = lambda tmpdir: f"local://{tmpdir}"
